# revision 2
# baseline (speedup 1.0000x reference)
"""Trainium2 Bass kernel for nn_BlockRevert.

Computation (per batch b, token s):
  out[b,s,0,:]   = temporal_block[b,s,0,:] + pe[s,:] + mod_emb[0,:]
  out[b,s,r+1,:] = (valid[b,s,idx] if idx<8 else mask_token) + pe[s,:] + mod_emb[r+1,:]
     where idx = revert_idx[b,s,r], valid[b,s,j] = temporal_block[b,s,1+j,:]

Sharding: data-parallel over batch, 1 batch per NeuronCore (8 cores).
Per core the gather is fully local. The host builds an interleaved table
with 10 rows per token (global, 8 valid, 1 mask-token copy): the
per-token mask copy spreads the ~2/3 mask-slot reads across all HBM
channels (a single shared mask row serializes on one channel), and any
revert index >= 8 is pointed at that token's mask row.

Device program per core (token-major layout, 4 blocks of 128 tokens),
with all index/pe loads hoisted to the front:
  tile t[128 tokens, 17*512], processed in slot-chunks (5,3,3,3,3):
    dma_gather chunk   (dst[p, j, :] = tbl[idx[j*128+p], :], 2KB rows)
    t_chunk += pe[s,:]   (broadcast along the slots via a step-0 AP)
    t_chunk += modrep    (mod_emb rows, host-replicated across partitions)
    store t_chunk
  Each chunk pipelines SDMA -> DVE -> SDMA independently; the big chunk
  goes first so its gather overlaps all downstream work.
"""

import os
import sys

import numpy as np

for _p in ("/opt/trn_rl_repo",):
    if _p not in sys.path and os.path.isdir(_p):
        sys.path.insert(0, _p)

B, S, MV, D, R = 8, 512, 8, 512, 16
NSLOT = R + 1          # 17 output slots
W = NSLOT * D          # 8704 floats per output row
NTR = MV + 2           # 10 table rows per token: global + 8 valid + mask copy
NT = S * NTR           # 5120 table rows per batch
BLK = 128              # tokens per block
NBLK = S // BLK
NIDX = BLK * NSLOT     # 2176 gathered rows per block (all 17 slots)
# slot-chunk boundaries shared by gather/add/store so each chunk pipelines
# through SDMA -> DVE -> SDMA independently
CHUNK_BOUNDS = (0, 5, 8, 11, 14, 17)  # big chunk first: its gather issues
# earliest and the smaller tail chunks shorten the end-of-kernel add+store

MODE = os.environ.get("BLOCKREVERT_MODE", "itl")


def _sinusoidal_pe(seq_len, d_model):
    pos = np.arange(seq_len)[:, None].astype(np.float32)
    div = np.exp(
        np.arange(0, d_model, 2).astype(np.float32) * (-np.log(10000.0) / d_model)
    )
    pe = np.zeros((seq_len, d_model), dtype=np.float32)
    pe[:, 0::2] = np.sin(pos * div)
    pe[:, 1::2] = np.cos(pos * div)
    return pe


def build_nc(mode=MODE, n_iter=None):
    import concourse.bacc as bacc
    import concourse.mybir as mybir
    import concourse.tile as tile

    f32 = mybir.dt.float32
    i16 = mybir.dt.int16

    nc = bacc.Bacc("TRN2", target_bir_lowering=False, debug=False)

    tbl = nc.dram_tensor("tbl", [NT, D], f32, kind="ExternalInput")
    # per-block dma_gather index buffers: wrapped into 16 partitions and
    # replicated across the 8 gpsimd cores -> [128, num_idxs/16] per block
    gidx = nc.dram_tensor("gidx", [NBLK * BLK, NIDX // 16], i16, kind="ExternalInput")
    pe = nc.dram_tensor("pe", [S, D], f32, kind="ExternalInput")
    modrep = nc.dram_tensor("modrep", [BLK, W], f32, kind="ExternalInput")
    out = nc.dram_tensor("out", [S, W], f32, kind="ExternalOutput")

    tbl_rows = tbl.ap()  # [NT, D]

    with tile.TileContext(nc) as tc:
        with (
            tc.tile_pool(name="const", bufs=1) as cpool,
            tc.tile_pool(name="work", bufs=3) as wpool,
            tc.tile_pool(name="small", bufs=3) as spool,
        ):

            def body():
                modt = cpool.tile([BLK, W], f32)
                nc.sync.dma_start(out=modt[:], in_=modrep.ap())

                # hoist all index/pe loads so gathers start immediately
                its, pts = [], []
                for i in range(NBLK):
                    it = spool.tile([BLK, NIDX // 16], i16, tag=f"it{i}")
                    pt = spool.tile([BLK, D], f32, tag=f"pt{i}")
                    nc.sync.dma_start(
                        out=it[:], in_=gidx.ap()[i * BLK : (i + 1) * BLK]
                    )
                    nc.sync.dma_start(out=pt[:], in_=pe.ap()[i * BLK : (i + 1) * BLK])
                    its.append(it)
                    pts.append(pt)

                for i in range(NBLK):
                    s0 = i * BLK
                    t = wpool.tile([BLK, W], f32)
                    it, pt = its[i], pts[i]
                    # per chunk: gather (dst[p, j, :] = tbl[idx[j*128+p], :]),
                    # then += pe (broadcast over slots), += mod, store
                    for ci in range(len(CHUNK_BOUNDS) - 1):
                        slo, shi = CHUNK_BOUNDS[ci], CHUNK_BOUNDS[ci + 1]
                        nsl = shi - slo
                        per = nsl * BLK
                        tv = t[:, slo * D : shi * D].rearrange(
                            "p (m d) -> p m d", d=D
                        )
                        nc.gpsimd.dma_gather(
                            out_ap=tv,
                            in_ap=tbl_rows,
                            idxs_ap=it[:, (slo * BLK) // 16 : (shi * BLK) // 16],
                            num_idxs=per,
                            num_idxs_reg=per,
                            elem_size=D,
                            single_packet=False,
                        )
                        pe_b = pt[:].unsqueeze(1).to_broadcast([BLK, nsl, D])
                        nc.vector.tensor_add(out=tv, in0=tv, in1=pe_b)
                        nc.vector.tensor_add(
                            out=t[:, slo * D : shi * D],
                            in0=t[:, slo * D : shi * D],
                            in1=modt[:, slo * D : shi * D],
                        )
                        nc.sync.dma_start(
                            out=out.ap()[s0 : s0 + BLK, slo * D : shi * D],
                            in_=t[:, slo * D : shi * D],
                        )

            if n_iter is None:
                body()
            else:
                with tc.For_i(0, n_iter):
                    body()

    nc.compile()
    return nc


def make_bench_arrays(rng, real_gidx=None):
    """Input arrays (one core's worth) for the bench repeat-loop."""
    gidx = real_gidx
    if gidx is None:
        g = rng.integers(0, NT, size=(NBLK, NSLOT * BLK), dtype=np.int16)
        gidx = np.ascontiguousarray(
            np.tile(
                g.reshape(NBLK, NIDX // 16, 16).transpose(0, 2, 1), (1, 8, 1)
            ).reshape(NBLK * BLK, NIDX // 16)
        )
    return {
        "tbl": rng.standard_normal((NT, D), dtype=np.float32),
        "gidx": gidx,
        "pe": rng.standard_normal((S, D), dtype=np.float32),
        "modrep": rng.standard_normal((BLK, W), dtype=np.float32),
    }


def make_in_maps(temporal_block, mask_token, mod_emb, revert_idx, mode=MODE):
    temporal_block = np.asarray(temporal_block, dtype=np.float32)
    mask_token = np.asarray(mask_token, dtype=np.float32)
    mod_emb = np.asarray(mod_emb, dtype=np.float32)
    revert_idx = np.asarray(revert_idx)

    pe = _sinusoidal_pe(S, D)
    modrep = np.ascontiguousarray(
        np.broadcast_to(mod_emb[:NSLOT].reshape(1, W), (BLK, W))
    )

    # interleaved table: rows s*10+m = temporal_block[s,m] for m<9,
    # row s*10+9 = mask token (per-token copy -> HBM channel balance)
    mask_col = np.broadcast_to(mask_token, (B, S, 1, D))
    tbl_all = np.concatenate([temporal_block, mask_col], axis=2).reshape(B, NT, D)

    # slot->table-row indices per token: [S, NSLOT]
    idx_all = revert_idx.astype(np.int64)  # [B, S, R]
    srow = np.arange(S, dtype=np.int64) * NTR  # [S]
    g_all = np.where(
        idx_all < MV, srow[None, :, None] + 1 + idx_all, srow[None, :, None] + MV + 1
    )
    g_full = np.concatenate(
        [np.broadcast_to(srow[None, :, None], (B, S, 1)), g_all], axis=2
    ).astype(np.int16)  # [B, S, NSLOT]

    in_maps = []
    for b in range(B):
        g = g_full[b]  # [S, NSLOT]
        # dma_gather order: dst[p, j] = tbl[idxk[j*128+p]] with idxk[k]
        # stored at idxs_sbuf[k % 16, k // 16], and the 16-partition block
        # replicated across all 8 gpsimd cores (128 partitions total).
        gw = np.empty((NBLK, BLK, NIDX // 16), dtype=np.int16)
        for i in range(NBLK):
            blk = g[i * BLK : (i + 1) * BLK]          # [128 tokens, 17 slots]
            idxk = blk.T.reshape(-1)                  # k = j*128 + p
            w16 = idxk.reshape(NIDX // 16, 16).T      # [16, num_idxs/16]
            gw[i] = np.tile(w16, (8, 1))              # replicate across cores
        in_maps.append(
            {
                "tbl": tbl_all[b],
                "gidx": np.ascontiguousarray(gw.reshape(NBLK * BLK, NIDX // 16)),
                "pe": pe,
                "modrep": modrep,
            }
        )
    return in_maps


_CACHE = {}


def _get_nc(mode=MODE):
    if mode not in _CACHE:
        _CACHE[mode] = build_nc(mode)
    return _CACHE[mode]


def kernel(temporal_block, mask_token, mod_emb, revert_idx):
    from concourse.bass_utils import run_bass_kernel_spmd

    nc = _get_nc()
    in_maps = make_in_maps(temporal_block, mask_token, mod_emb, revert_idx)
    res = run_bass_kernel_spmd(nc, in_maps, core_ids=list(range(B)))
    out = np.stack([res.results[b]["out"].reshape(S, NSLOT, D) for b in range(B)])
    return out

